# revision 12
# baseline (speedup 1.0000x reference)
"""LoRALinear kernel for Trainium2 (8 NeuronCores, data-parallel over tokens).

Math: out = x @ W.T + b + s1*(x@A1.T)@B1.T + s2*(x@A2.T)@B2.T
    = x @ (W + s1*B1@A1 + s2*B2@A2).T + b

The LoRA adapters are folded into the base weight on-device (rank-16 fold is
tiny), turning the whole problem into one dense [T,1024]@[1024,1024] matmul
plus a broadcast bias add. x is sharded 4096 tokens per core; all weights are
replicated; no collectives.

Sharding/layout choice (host side, pure layout transforms only): x is passed
per-core as x.T columns so the contraction dim lands on SBUF partitions with
fast contiguous DMA; W/B1/B2 are passed transposed for the same reason. All
arithmetic (scaling, LoRA fold, matmul, bias) runs on device.

Per-core pipeline:
  prep : DMA W.T, round to fp32r (DVE), fold s1*A1.T@B1.T + s2*A2.T@B2.T via
         two rank-16 PE matmuls per tile + DVE add; DMA-broadcast bias.
  main : per 128-token tile: DMA xT tile, DVE fp32r rounding copy,
         8 accumulating fp32r matmuls per 512-wide psum, DVE bias-add, DMA out.
"""

import sys

import numpy as np

try:
    import concourse.bass as bass
except ImportError:
    sys.path.insert(0, "/opt/trn_rl_repo")
    import concourse.bass as bass

from concourse import bacc

import concourse.mybir as mybir
import concourse.tile as tile
from concourse.bass_utils import run_bass_kernel_spmd

TOKENS, D, RANK = 32768, 1024, 16
N_CORES = 8
T_SHARD = TOKENS // N_CORES  # 4096
SCALE1 = 8.0 / RANK
SCALE2 = 16.0 / RANK
F32 = mybir.dt.float32
F32R = mybir.dt.float32r
P = 128
N_TT = T_SHARD // P  # 32 token tiles per core
N_IC = D // P  # 8 contraction chunks
N_OC = D // 512  # 2 psum-wide output chunks


def build_nc():
    nc = bacc.Bacc("TRN2")
    xT = nc.dram_tensor("xT", [D, T_SHARD], F32, kind="ExternalInput")
    WT = nc.dram_tensor("WT", [D, D], F32, kind="ExternalInput")
    b = nc.dram_tensor("b", [D], F32, kind="ExternalInput")
    A1 = nc.dram_tensor("A1", [RANK, D], F32, kind="ExternalInput")
    B1T = nc.dram_tensor("B1T", [RANK, D], F32, kind="ExternalInput")
    A2 = nc.dram_tensor("A2", [RANK, D], F32, kind="ExternalInput")
    B2T = nc.dram_tensor("B2T", [RANK, D], F32, kind="ExternalInput")
    out = nc.dram_tensor("out", [T_SHARD, D], F32, kind="ExternalOutput")

    with tile.TileContext(nc) as tc:
        with (
            tc.tile_pool(name="const", bufs=1) as const,
            tc.tile_pool(name="xp", bufs=6) as xpool,
            tc.tile_pool(name="xtp", bufs=6) as xtpool,
            tc.tile_pool(name="op", bufs=6) as opool,
            tc.tile_pool(name="psm", bufs=6, space="PSUM") as psum_m,
        ):
            # bias broadcast across all 128 partitions (tokens sit on partitions)
            bias_sb = const.tile([P, D], F32)
            b_ap = b[:]
            bias_bcast = bass.AP(
                tensor=b_ap.tensor, offset=b_ap.offset, ap=[[0, P], [1, D]]
            )
            nc.sync.dma_start(out=bias_sb, in_=bias_bcast)

            # W.T layout [i_inner(128), i_outer(8), o(1024)], rounded to fp32r
            WT_ld = const.tile([P, N_IC, D], F32)
            nc.sync.dma_start(WT_ld, WT[:].rearrange("(io ii) o -> ii io o", ii=P))
            WT_sb = const.tile([P, N_IC, D], F32R)
            for io in range(N_IC):
                nc.vector.tensor_copy(out=WT_sb[:, io, :], in_=WT_ld[:, io, :])

            # adapters (A natural, B pre-transposed on host; scales on device)
            A1_ld = const.tile([RANK, D], F32)
            nc.sync.dma_start(A1_ld, A1[:])
            A2_ld = const.tile([RANK, D], F32)
            nc.sync.dma_start(A2_ld, A2[:])
            A1_sb = const.tile([RANK, D], F32R)
            nc.vector.tensor_copy(out=A1_sb, in_=A1_ld)
            A2_sb = const.tile([RANK, D], F32R)
            nc.vector.tensor_copy(out=A2_sb, in_=A2_ld)

            B1T_ld = const.tile([RANK, D], F32)
            nc.sync.dma_start(B1T_ld, B1T[:])
            B2T_ld = const.tile([RANK, D], F32)
            nc.sync.dma_start(B2T_ld, B2T[:])
            B1T_sb = const.tile([RANK, D], F32R)
            nc.vector.tensor_scalar_mul(B1T_sb, B1T_ld, SCALE1)
            B2T_sb = const.tile([RANK, D], F32R)
            nc.vector.tensor_scalar_mul(B2T_sb, B2T_ld, SCALE2)

            # fold LoRA: WT += s1*A1.T@B1.T + s2*A2.T@B2.T (scales baked in BT)
            for ic in range(N_IC):
                for on in range(N_OC):
                    psd = psum_m.tile([P, 512], F32, tag="psd")
                    nc.tensor.matmul(
                        psd,
                        lhsT=A1_sb[:, ic * P : (ic + 1) * P],
                        rhs=B1T_sb[:, on * 512 : (on + 1) * 512],
                        start=True,
                        stop=False,
                    )
                    nc.tensor.matmul(
                        psd,
                        lhsT=A2_sb[:, ic * P : (ic + 1) * P],
                        rhs=B2T_sb[:, on * 512 : (on + 1) * 512],
                        start=False,
                        stop=True,
                    )
                    nc.vector.tensor_add(
                        out=WT_sb[:, ic, on * 512 : (on + 1) * 512],
                        in0=WT_sb[:, ic, on * 512 : (on + 1) * 512].bitcast(F32),
                        in1=psd,
                    )

            # main loop: 32 token tiles of 128
            for tt in range(N_TT):
                x_ld = xpool.tile([P, N_IC, P], F32, tag="x")
                nc.sync.dma_start(
                    x_ld,
                    xT[:, tt * P : (tt + 1) * P].rearrange(
                        "(io ii) t -> ii io t", ii=P
                    ),
                )
                xT_sb = xtpool.tile([P, N_IC, P], F32R, tag="xt")
                nc.vector.tensor_copy(out=xT_sb, in_=x_ld)
                o_sb = opool.tile([P, D], F32, tag="o")
                for on in range(N_OC):
                    pso = psum_m.tile([P, 512], F32, tag="psd")
                    for ic in range(N_IC):
                        nc.tensor.matmul(
                            pso,
                            lhsT=xT_sb[:, ic, :],
                            rhs=WT_sb[:, ic, on * 512 : (on + 1) * 512],
                            start=(ic == 0),
                            stop=(ic == N_IC - 1),
                        )
                    nc.vector.tensor_add(
                        out=o_sb[:, on * 512 : (on + 1) * 512],
                        in0=pso,
                        in1=bias_sb[:, on * 512 : (on + 1) * 512],
                    )
                nc.sync.dma_start(out[tt * P : (tt + 1) * P, :], o_sb)

    nc.finalize()
    return nc


_NC = None


def _get_nc():
    global _NC
    if _NC is None:
        _NC = build_nc()
    return _NC


def kernel(**inputs):
    x = np.asarray(inputs["x"], dtype=np.float32)
    shared = {
        "WT": np.ascontiguousarray(np.asarray(inputs["W"], np.float32).T),
        "b": np.ascontiguousarray(np.asarray(inputs["b"], np.float32)),
        "A1": np.ascontiguousarray(np.asarray(inputs["A1"], np.float32)),
        "B1T": np.ascontiguousarray(np.asarray(inputs["B1"], np.float32).T),
        "A2": np.ascontiguousarray(np.asarray(inputs["A2"], np.float32)),
        "B2T": np.ascontiguousarray(np.asarray(inputs["B2"], np.float32).T),
    }
    in_maps = []
    for c in range(N_CORES):
        m = dict(shared)
        m["xT"] = np.ascontiguousarray(x[c * T_SHARD : (c + 1) * T_SHARD].T)
        in_maps.append(m)
    res = run_bass_kernel_spmd(_get_nc(), in_maps, core_ids=list(range(N_CORES)))
    return np.concatenate([r["out"] for r in res.results], axis=0)


# revision 13
# speedup vs baseline: 1.0131x; 1.0131x over previous
"""LoRALinear kernel for Trainium2 (8 NeuronCores, data-parallel over tokens).

Math: out = x @ W.T + b + s1*(x@A1.T)@B1.T + s2*(x@A2.T)@B2.T
    = x @ (W + s1*B1@A1 + s2*B2@A2).T + b

The LoRA adapters are folded into the base weight on-device (rank-16 fold is
tiny), turning the whole problem into one dense [T,1024]@[1024,1024] matmul
plus a broadcast bias add. x is sharded 4096 tokens per core; all weights are
replicated; no collectives.

Sharding/layout choice (host side, pure layout transforms only): x is passed
per-core as x.T columns so the contraction dim lands on SBUF partitions with
fast contiguous DMA; W/B1/B2 are passed transposed for the same reason. All
arithmetic (scaling, LoRA fold, matmul, bias) runs on device.

Per-core pipeline:
  prep : DMA W.T, round to fp32r (DVE), fold s1*A1.T@B1.T + s2*A2.T@B2.T via
         two rank-16 PE matmuls per tile + DVE add; DMA-broadcast bias.
  main : per 128-token tile: DMA xT tile, DVE fp32r rounding copy,
         8 accumulating fp32r matmuls per 512-wide psum, DVE bias-add, DMA out.
"""

import sys

import numpy as np

try:
    import concourse.bass as bass
except ImportError:
    sys.path.insert(0, "/opt/trn_rl_repo")
    import concourse.bass as bass

from concourse import bacc

import concourse.mybir as mybir
import concourse.tile as tile
from concourse.bass_utils import run_bass_kernel_spmd

TOKENS, D, RANK = 32768, 1024, 16
N_CORES = 8
T_SHARD = TOKENS // N_CORES  # 4096
SCALE1 = 8.0 / RANK
SCALE2 = 16.0 / RANK
F32 = mybir.dt.float32
F32R = mybir.dt.float32r
P = 128
N_TT = T_SHARD // P  # 32 token tiles per core
N_IC = D // P  # 8 contraction chunks
N_OC = D // 512  # 2 psum-wide output chunks


def build_nc():
    nc = bacc.Bacc("TRN2")
    xT = nc.dram_tensor("xT", [D, T_SHARD], F32, kind="ExternalInput")
    WT = nc.dram_tensor("WT", [D, D], F32, kind="ExternalInput")
    b = nc.dram_tensor("b", [D], F32, kind="ExternalInput")
    A1 = nc.dram_tensor("A1", [RANK, D], F32, kind="ExternalInput")
    B1T = nc.dram_tensor("B1T", [RANK, D], F32, kind="ExternalInput")
    A2 = nc.dram_tensor("A2", [RANK, D], F32, kind="ExternalInput")
    B2T = nc.dram_tensor("B2T", [RANK, D], F32, kind="ExternalInput")
    out = nc.dram_tensor("out", [T_SHARD, D], F32, kind="ExternalOutput")

    with tile.TileContext(nc) as tc:
        with (
            tc.tile_pool(name="const", bufs=1) as const,
            tc.tile_pool(name="xp", bufs=4) as xpool,
            tc.tile_pool(name="xtp", bufs=4) as xtpool,
            tc.tile_pool(name="op", bufs=4) as opool,
            tc.tile_pool(name="psm", bufs=4, space="PSUM") as psum_m,
        ):
            # bias broadcast across all 128 partitions (tokens sit on partitions)
            bias_sb = const.tile([P, D], F32)
            b_ap = b[:]
            bias_bcast = bass.AP(
                tensor=b_ap.tensor, offset=b_ap.offset, ap=[[0, P], [1, D]]
            )
            nc.sync.dma_start(out=bias_sb, in_=bias_bcast)

            # W.T layout [i_inner(128), i_outer(8), o(1024)], rounded to fp32r
            WT_ld = const.tile([P, N_IC, D], F32)
            nc.sync.dma_start(WT_ld, WT[:].rearrange("(io ii) o -> ii io o", ii=P))
            WT_sb = const.tile([P, N_IC, D], F32R)
            for io in range(N_IC):
                nc.vector.tensor_copy(out=WT_sb[:, io, :], in_=WT_ld[:, io, :])

            # adapters (A natural, B pre-transposed on host; scales on device)
            A1_ld = const.tile([RANK, D], F32)
            nc.sync.dma_start(A1_ld, A1[:])
            A2_ld = const.tile([RANK, D], F32)
            nc.sync.dma_start(A2_ld, A2[:])
            A1_sb = const.tile([RANK, D], F32R)
            nc.vector.tensor_copy(out=A1_sb, in_=A1_ld)
            A2_sb = const.tile([RANK, D], F32R)
            nc.vector.tensor_copy(out=A2_sb, in_=A2_ld)

            B1T_ld = const.tile([RANK, D], F32)
            nc.sync.dma_start(B1T_ld, B1T[:])
            B2T_ld = const.tile([RANK, D], F32)
            nc.sync.dma_start(B2T_ld, B2T[:])
            B1T_sb = const.tile([RANK, D], F32R)
            nc.vector.tensor_scalar_mul(B1T_sb, B1T_ld, SCALE1)
            B2T_sb = const.tile([RANK, D], F32R)
            nc.vector.tensor_scalar_mul(B2T_sb, B2T_ld, SCALE2)

            # fold LoRA: WT += s1*A1.T@B1.T + s2*A2.T@B2.T (scales baked in BT)
            for ic in range(N_IC):
                for on in range(N_OC):
                    psd = psum_m.tile([P, 512], F32, tag="psd")
                    nc.tensor.matmul(
                        psd,
                        lhsT=A1_sb[:, ic * P : (ic + 1) * P],
                        rhs=B1T_sb[:, on * 512 : (on + 1) * 512],
                        start=True,
                        stop=False,
                    )
                    nc.tensor.matmul(
                        psd,
                        lhsT=A2_sb[:, ic * P : (ic + 1) * P],
                        rhs=B2T_sb[:, on * 512 : (on + 1) * 512],
                        start=False,
                        stop=True,
                    )
                    nc.vector.tensor_add(
                        out=WT_sb[:, ic, on * 512 : (on + 1) * 512],
                        in0=WT_sb[:, ic, on * 512 : (on + 1) * 512].bitcast(F32),
                        in1=psd,
                    )

            # main loop: 32 token tiles of 128
            for tt in range(N_TT):
                x_ld = xpool.tile([P, N_IC, P], F32, tag="x")
                nc.sync.dma_start(
                    x_ld,
                    xT[:, tt * P : (tt + 1) * P].rearrange(
                        "(io ii) t -> ii io t", ii=P
                    ),
                )
                xT_sb = xtpool.tile([P, N_IC, P], F32R, tag="xt")
                nc.vector.tensor_copy(out=xT_sb, in_=x_ld)
                o_sb = opool.tile([P, D], F32, tag="o")
                for on in range(N_OC):
                    pso = psum_m.tile([P, 512], F32, tag="psd")
                    for ic in range(N_IC):
                        nc.tensor.matmul(
                            pso,
                            lhsT=xT_sb[:, ic, :],
                            rhs=WT_sb[:, ic, on * 512 : (on + 1) * 512],
                            start=(ic == 0),
                            stop=(ic == N_IC - 1),
                        )
                    nc.vector.tensor_add(
                        out=o_sb[:, on * 512 : (on + 1) * 512],
                        in0=pso,
                        in1=bias_sb[:, on * 512 : (on + 1) * 512],
                    )
                nc.sync.dma_start(out[tt * P : (tt + 1) * P, :], o_sb)

    nc.finalize()
    return nc


_NC = None


def _get_nc():
    global _NC
    if _NC is None:
        _NC = build_nc()
    return _NC


def kernel(**inputs):
    x = np.asarray(inputs["x"], dtype=np.float32)
    shared = {
        "WT": np.ascontiguousarray(np.asarray(inputs["W"], np.float32).T),
        "b": np.ascontiguousarray(np.asarray(inputs["b"], np.float32)),
        "A1": np.ascontiguousarray(np.asarray(inputs["A1"], np.float32)),
        "B1T": np.ascontiguousarray(np.asarray(inputs["B1"], np.float32).T),
        "A2": np.ascontiguousarray(np.asarray(inputs["A2"], np.float32)),
        "B2T": np.ascontiguousarray(np.asarray(inputs["B2"], np.float32).T),
    }
    in_maps = []
    for c in range(N_CORES):
        m = dict(shared)
        m["xT"] = np.ascontiguousarray(x[c * T_SHARD : (c + 1) * T_SHARD].T)
        in_maps.append(m)
    res = run_bass_kernel_spmd(_get_nc(), in_maps, core_ids=list(range(N_CORES)))
    return np.concatenate([r["out"] for r in res.results], axis=0)
